# revision 27
# baseline (speedup 1.0000x reference)
"""MoE (8 experts, top-2) Trainium2 kernel — v2.

Strategy (per spec sharding_hint): expert parallelism. The host computes the
(cheap) router — logits, softmax, top-2, renormalized combine weights — and
dispatches each token to the cores owning its two experts ("all-to-all token
dispatch by top-k expert id" done at the sharding step, since kernel() holds
the full inputs host-side). Core e runs the expert-e FFN over its gathered
tokens, capacity-padded so all 8 cores run one SPMD program.

v2 layout (vs v1): everything bf16, the gelu intermediate h stays in SBUF
(no DRAM round-trip), and I is processed in Q=4 slices of 1024 so one
slice's h fits in SBUF; each slice emits a partial y the host sums. Within
a slice, token tiles are processed in groups of 4 so each loaded stationary
weight (128x128) serves 4 matmuls — amortizing the unoverlapped LDWEIGHTS
(~53-107ns/MM otherwise, measured as the main gap vs the cost model in v1):

    h_q[i,t] = gelu(W1[:,i]^T XT + b1[i]),  i in slice q   (psum f32)
    Yq[o,t]  = sum_{i in q} W2[i,o] h_q[i,t]               (psum f32)
"""

import os
import sys

import numpy as np

for _p in ("/opt/trn_rl_repo", "/root/.axon_site/_ro/trn_rl_repo"):
    if os.path.isdir(_p) and _p not in sys.path:
        sys.path.insert(0, _p)

NUM_EXPERTS = 8
TOP_K = 2
B, S, H, I = 4, 4096, 1024, 4096
T = B * S
P = 128
NT = 512           # max token tile
C_DEFAULT = 4224   # capacity per expert (seed-0 max count 4181), mult of 128
KH = H // P        # 8 contraction chunks for stage 1
Q = 4              # I-slices
IQ = I // Q        # 1024 i-values per slice
NB = IQ // P       # 8 i-blocks (and stage-2 k-chunks) per slice
NO = H // P        # 8 output blocks
G = 4              # token tiles per weight-reuse group (psum-bank bound)

_built = {}        # (C, reps) -> nc


def _token_tiles(C):
    """Split C into tiles of 512 plus at most one trailing 128/256/384."""
    assert C % 128 == 0
    tiles, off = [], 0
    while C - off >= 512:
        tiles.append((off, 512))
        off += 512
    if C - off:
        tiles.append((off, C - off))
        off = C
    return tiles


def _build(C, reps=1):
    import concourse.bacc as bacc
    import concourse.mybir as mybir
    import concourse.tile as tile
    from concourse._compat import get_trn_type

    f32 = mybir.dt.float32
    bf16 = mybir.dt.bfloat16
    GELU = mybir.ActivationFunctionType.Gelu

    nc = bacc.Bacc(
        get_trn_type() or "TRN2",
        target_bir_lowering=False,
        debug=False,
        enable_asserts=False,
    )
    xt = nc.dram_tensor("xt", [H, C], bf16, kind="ExternalInput").ap()
    w1 = nc.dram_tensor("w1", [P, KH, I], bf16, kind="ExternalInput").ap()
    b1 = nc.dram_tensor("b1", [I], f32, kind="ExternalInput").ap()
    w2 = nc.dram_tensor("w2", [Q * NO, P, NB, P], bf16,
                        kind="ExternalInput").ap()
    y = nc.dram_tensor("y", [Q, H, C], bf16, kind="ExternalOutput").ap()

    tiles = _token_tiles(C)
    groups = [tiles[i:i + G] for i in range(0, len(tiles), G)]

    with tile.TileContext(nc) as tc:
        with (
            tc.tile_pool(name="bias", bufs=1) as bpool,
            tc.tile_pool(name="w1p", bufs=1) as w1p,
            tc.tile_pool(name="w2p", bufs=2) as w2p,
            tc.tile_pool(name="xp", bufs=1) as xp,
            tc.tile_pool(name="hp", bufs=1) as hp,
            tc.tile_pool(name="yp", bufs=3) as yp,
            tc.tile_pool(name="psp", bufs=8, space="PSUM") as psp,
        ):
            b1sb = bpool.tile([P, I // P], f32)
            nc.sync.dma_start(b1sb[:], b1.rearrange("(ib p) -> p ib", p=P))

            def _load_w1(rep, q, split=False):
                w1sb = w1p.tile([P, KH, IQ], bf16, tag="w1",
                                name=f"w1_{rep}_{q}")
                lo = q * IQ
                if split:  # halves, so the first i-blocks land sooner
                    hw = IQ // 2
                    nc.sync.dma_start(w1sb[:, :, 0:hw], w1[:, :, lo:lo + hw])
                    nc.sync.dma_start(w1sb[:, :, hw:IQ],
                                      w1[:, :, lo + hw:lo + IQ])
                else:
                    nc.sync.dma_start(w1sb[:], w1[:, :, lo:lo + IQ])
                return w1sb

            def _load_w2(rep, q):
                # all 8 o-block slices of this q-slice's w2, one DMA,
                # prefetched during stage 1
                w2sb = w2p.tile([P, NO, NB, P], bf16, tag="w2",
                                name=f"w2_{rep}_{q}")
                nc.sync.dma_start(
                    w2sb[:],
                    w2[q * NO:(q + 1) * NO].rearrange("a p b c -> p a b c"),
                )
                return w2sb

            for rep in range(reps):
                # Prologue order: w1(q0) first so PE can start ~6us in, then
                # the first group's x tiles, w2(q0), then the rest of x.
                # x stays resident in SBUF for the whole rep.
                w1sb0 = _load_w1(rep, 0, split=True)
                xall = xp.tile([P, KH, C], bf16, tag="xa", name=f"xa_{rep}")

                def _load_x(toff, tsz):
                    nc.sync.dma_start(
                        xall[:, :, toff:toff + tsz],
                        xt[:, toff:toff + tsz].rearrange(
                            "(ko p) n -> p ko n", p=P),
                    )
                for toff, tsz in tiles[:G]:
                    _load_x(toff, tsz)
                w2sb0 = _load_w2(rep, 0)
                for toff, tsz in tiles[G:]:
                    _load_x(toff, tsz)

                for q in range(Q):
                    w1sb = w1sb0 if q == 0 else _load_w1(rep, q)
                    w2sb = w2sb0 if q == 0 else _load_w2(rep, q)
                    h = hp.tile([P, NB, C], bf16, tag="h",
                                name=f"h_{rep}_{q}")

                    # -- stage 1: h = gelu(w1q^T x + b1q) over this I-slice
                    for g, tg in enumerate(groups):
                        for ib in range(NB):
                            pss = [
                                psp.tile([P, tsz], f32, tag="ps",
                                         name=f"ps1_{rep}_{q}_{g}_{ib}_{ti}")
                                for ti, (toff, tsz) in enumerate(tg)
                            ]
                            # k_inner: stationary weight reused across the
                            # group's token tiles (LDWEIGHTS amortization
                            # beats psum-bank-switch cost; A/B-measured).
                            for k in range(KH):
                                for ti, (toff, tsz) in enumerate(tg):
                                    nc.tensor.matmul(
                                        pss[ti][:],
                                        lhsT=w1sb[:, k, ib * P:(ib + 1) * P],
                                        rhs=xall[:, k, toff:toff + tsz],
                                        start=(k == 0),
                                        stop=(k == KH - 1),
                                    )
                            ibg = q * NB + ib
                            for ti, (toff, tsz) in enumerate(tg):
                                nc.scalar.activation(
                                    h[:, ib, toff:toff + tsz], pss[ti][:],
                                    GELU, bias=b1sb[:, ibg:ibg + 1],
                                )

                    # -- stage 2: y[q] = w2q^T h  (partial over this I-slice)
                    for ob in range(NO):
                        for g, tg in enumerate(groups):
                            gstart = tg[0][0]
                            gsz = sum(tsz for _, tsz in tg)
                            pss = [
                                psp.tile([P, tsz], f32, tag="ps",
                                         name=f"ps2_{rep}_{q}_{ob}_{g}_{ti}")
                                for ti, (toff, tsz) in enumerate(tg)
                            ]
                            for kk in range(NB):
                                for ti, (toff, tsz) in enumerate(tg):
                                    nc.tensor.matmul(
                                        pss[ti][:],
                                        lhsT=w2sb[:, ob, kk],
                                        rhs=h[:, kk, toff:toff + tsz],
                                        start=(kk == 0),
                                        stop=(kk == NB - 1),
                                    )
                            # batch the group's drains into one SBUF tile,
                            # one DMA (on the ACT hwdge ring — SP carries
                            # x/w traffic)
                            yg = yp.tile([P, gsz], bf16, tag="y",
                                         name=f"y_{rep}_{q}_{ob}_{g}")
                            for ti, (toff, tsz) in enumerate(tg):
                                o = toff - gstart
                                nc.vector.tensor_copy(
                                    yg[:, o:o + tsz], pss[ti][:])
                            nc.scalar.dma_start(
                                y[q, ob * P:(ob + 1) * P,
                                  gstart:gstart + gsz],
                                yg[:],
                            )
    nc.finalize()
    return nc


def _routing(hidden, router_w, router_b):
    """Top-2 routing, bit-matching the jax reference on CPU."""
    import jax
    import jax.numpy as jnp

    cpu = jax.local_devices(backend="cpu")[0]
    with jax.default_device(cpu):
        logits = jnp.einsum("bsh,he->bse", jnp.asarray(hidden),
                            jnp.asarray(router_w)) + jnp.asarray(router_b)
        probs = jax.nn.softmax(logits, axis=-1)
        tkp, tki = jax.lax.top_k(probs, TOP_K)
        tkp = tkp / jnp.sum(tkp, axis=-1, keepdims=True)
        tkp_np = np.asarray(tkp).reshape(T, TOP_K)
        tki_np = np.asarray(tki).reshape(T, TOP_K)
    return tkp_np, tki_np


def _prepare(inputs):
    """Routing + per-expert input maps. Returns (C, in_maps, idx_e, prob_e)."""
    import ml_dtypes

    bf16 = ml_dtypes.bfloat16
    hidden_states = np.ascontiguousarray(
        inputs["hidden_states"], dtype=np.float32
    )
    w1 = np.ascontiguousarray(inputs["w1"], dtype=np.float32)
    b1 = np.ascontiguousarray(inputs["b1"], dtype=np.float32)
    w2 = np.ascontiguousarray(inputs["w2"], dtype=np.float32)

    tkp, tki = _routing(hidden_states, inputs["router_w"], inputs["router_b"])
    x = hidden_states.reshape(T, H)

    idx_e, prob_e = [], []
    for e in range(NUM_EXPERTS):
        hit = tki == e                       # [T, 2] bool
        idx = np.nonzero(hit.any(axis=1))[0]
        pe = np.where(hit[idx, 0], tkp[idx, 0], tkp[idx, 1]).astype(np.float32)
        idx_e.append(idx)
        prob_e.append(pe)

    maxn = max(len(ix) for ix in idx_e)
    C = C_DEFAULT if maxn <= C_DEFAULT else ((maxn + 127) // 128) * 128

    # w1 packed [E, P, KH, I]: w1p[e, p, k, i] = w1[e, k*P+p, i]
    w1p = np.ascontiguousarray(
        w1.reshape(NUM_EXPERTS, KH, P, I).transpose(0, 2, 1, 3)
    ).astype(bf16)
    # w2 packed [E, Q*NO, P, NB, P]:
    #   w2p[e, q*NO+ob, p, kk, o'] = w2[e, q*IQ + kk*P + p, ob*P + o']
    w2p = np.ascontiguousarray(
        w2.reshape(NUM_EXPERTS, Q, NB, P, NO, P).transpose(0, 1, 4, 3, 2, 5)
        .reshape(NUM_EXPERTS, Q * NO, P, NB, P)
    ).astype(bf16)

    in_maps = []
    for e in range(NUM_EXPERTS):
        ix = idx_e[e]
        xt = np.zeros((H, C), dtype=bf16)
        xt[:, :len(ix)] = x[ix].T.astype(bf16)
        in_maps.append({
            "xt": xt,
            "w1": w1p[e],
            "b1": b1[e],
            "w2": w2p[e],
        })
    return C, in_maps, idx_e, prob_e


def kernel(hidden_states, w1, b1, w2, b2, router_w, router_b):
    from concourse import bass_utils

    b2 = np.ascontiguousarray(b2, dtype=np.float32)
    C, in_maps, idx_e, prob_e = _prepare({
        "hidden_states": hidden_states, "w1": w1, "b1": b1, "w2": w2,
        "router_w": router_w, "router_b": router_b,
    })
    if C not in _built:
        _built[C] = _build(C)
    nc = _built[C]

    res = bass_utils.run_bass_kernel_spmd(
        nc, in_maps, core_ids=list(range(NUM_EXPERTS))
    ).results

    out = np.zeros((T, H), dtype=np.float32)
    for e in range(NUM_EXPERTS):
        ix = idx_e[e]
        yq = res[e]["y"]                     # [Q, H, C] bf16
        yf = yq[:, :, :len(ix)].astype(np.float32).sum(axis=0)
        out[ix] += (yf.T + b2[e]) * prob_e[e][:, None]
    return out.reshape(B, S, H)
